# revision 1
# baseline (speedup 1.0000x reference)
"""Multi-head causal self-attention with RoPE for Trainium2 (8 NeuronCores).

Problem: B=4, T=2048, C=1024, H=16 heads, D=64, fused QKV + causal softmax
attention + out-projection, fp32 I/O.

Sharding (Megatron-style): core c -> batch b = c//2, heads [8*(c%2), +8).
Each core computes its 8 heads' attention for its batch and a row-parallel
partial of the out-projection; the host sums the two partials per batch.

Per-core design:
  - All matmuls in bf16 (full PE rate; inputs quantized host-side).
  - q/k stored head-major: head h owns 64 contiguous partitions
    [32 even-rot | 32 odd-rot] -> scores are ONE K=64 matmul per head.
    RoPE projects through separate even/odd-dim tiles so the rotation
    combines read equal partition bases (HW SB+SB constraint) and write
    shifted output partitions.
  - W_qkv/W_v/W_out resident in SBUF (loaded once).
  - causal trim at exact 128-block granularity (bf16 matmuls have no
    narrow-free-dim rate penalty).
  - v carries 64 replicated ones-columns: attention*V lands the softmax
    denominators pre-broadcast on partitions 64-127 (no partition
    broadcast on the normalize path).
  - PSUM: scores 2x2-bank slots + 2 banks attn*V + 2 banks projections.
  - chunk j+1 projections are emitted interleaved at chunk j's sweep
    boundaries; reps pipeline across the boundary for repeat>1 timing.
"""

import numpy as np

B, T, C = 4, 2048, 1024
H, D = 16, 64
HC = 8               # heads per core
N_CORES = 8
THETA = 10000.0
NJ = T // 512        # 4 query/column chunks
NCT = C // 128       # 8 contraction tiles for projections

_CACHE = {}


def _build_program(repeat=1):
    """Build the per-core program. repeat>1 replays the whole computation
    (same inputs/outputs) for clean wall-clock timing."""
    import contextlib
    import concourse.tile as tile
    import concourse.mybir as mybir
    from concourse import bacc

    f32 = mybir.dt.float32
    f32r = mybir.dt.float32r
    bf16 = mybir.dt.bfloat16
    EXP = mybir.ActivationFunctionType.Exp
    MUL = mybir.AluOpType.mult
    SUB = mybir.AluOpType.subtract
    ADD = mybir.AluOpType.add

    nc = bacc.Bacc("TRN2", target_bir_lowering=False, debug=False)
    xT_t = nc.dram_tensor("xT", [C, T], bf16, kind="ExternalInput")
    wqk_t = nc.dram_tensor("wqk", [128, 8, NCT, 128], bf16, kind="ExternalInput")
    wv_t = nc.dram_tensor("wv", [C, HC * D], bf16, kind="ExternalInput")
    wout_t = nc.dram_tensor("wout", [HC * D, C], bf16, kind="ExternalInput")
    csn_t = nc.dram_tensor("csn", [128, 2, T], bf16, kind="ExternalInput")
    mk_t = nc.dram_tensor("mk", [128, 256], bf16, kind="ExternalInput")
    y_t = nc.dram_tensor("y", [T, C], f32, kind="ExternalOutput")

    with tile.TileContext(nc) as tc:
        with contextlib.ExitStack() as ctx:
            singles = ctx.enter_context(tc.tile_pool(name="singles", bufs=1))
            psum = ctx.enter_context(tc.tile_pool(name="psum", bufs=1, space="PSUM"))
            work = ctx.enter_context(tc.tile_pool(name="work", bufs=1))

            # ---- resident tensors -------------------------------------------
            kT_sb = singles.tile([128, 4, T], bf16, name="kT_sb")
            v_sb = singles.tile([128, T // 128, HC, 2 * D], bf16, name="v_sb")
            wqk_sb = singles.tile([128, 8, NCT, 128], bf16, name="wqk_sb")
            wv_sb = singles.tile([128, NCT, HC * D], bf16, name="wv_sb")
            wout_sb = singles.tile([128, 4, C], bf16, name="wout_sb")
            mk_sb = singles.tile([128, 256], bf16, name="mk_sb")

            # 64 replicated ones-columns: AV lands softmax denominators
            # pre-broadcast on partitions 64-127 (output partitions are free)
            nc.gpsimd.memset(v_sb[:, :, :, D:2 * D], 1.0)
            for g2 in range(4):
                nc.sync.dma_start(wqk_sb[:, 2 * g2:2 * g2 + 2],
                                  wqk_t.ap()[:, 2 * g2:2 * g2 + 2])
            nc.sync.dma_start(mk_sb[:], mk_t.ap())
            nc.sync.dma_start(wv_sb[:],
                              wv_t.ap().rearrange("(kt p) n -> p kt n", p=128))
            nc.sync.dma_start(wout_sb[:],
                              wout_t.ap().rearrange("(ct p) n -> p ct n", p=128))

            # per-chunk state (created by the proj steps, used by phase 2);
            # keyed (rep, j) so projections pipeline across rep boundaries
            qTc_ = {}
            css_ = {}

            def make_proj_steps(rep, j):
                """Micro-step closures for chunk (rep, j)'s projections+rope:
                [dmas] + per head-quad [even-proj, odd-proj, combines] + v."""
                c0 = 512 * j
                xtn = {}
                tcs_ = {}
                steps = []

                def dmas():
                    css = work.tile([128, 2, 512], bf16, tag="csn", bufs=3,
                                    name=f"r{rep}_csn{j}")
                    nc.sync.dma_start(css[:], csn_t.ap()[:, :, c0:c0 + 512])
                    css_[rep, j] = css
                    xt = work.tile([128, NCT, 512], bf16, tag="xtn", bufs=3,
                                   name=f"r{rep}_xtn{j}")
                    src = xT_t.ap().rearrange("(kt p) n -> p kt n", p=128)
                    nc.sync.dma_start(xt[:, 0:4], src[:, 0:4, c0:c0 + 512])
                    nc.sync.dma_start(xt[:, 4:8], src[:, 4:8, c0:c0 + 512])
                    xtn["t"] = xt
                    qTc_[rep, j] = work.tile([128, 4, 512], bf16, tag="qTc", bufs=2,
                                             name=f"r{rep}_qTc{j}")
                steps.append(dmas)

                def parblock(pr, par):
                    # one 128-dim projection tile (even or odd dims of a quad)
                    def run():
                        css = css_[rep, j]
                        xt = xtn["t"]
                        g = 2 * pr + par
                        pg = psum.tile([128, 512], f32, tag="pp", bufs=2,
                                       name=f"r{rep}_pg{j}_{g}")
                        for k in range(NCT):
                            nc.tensor.matmul(pg[:], wqk_sb[:, g, k, :], xt[:, k],
                                             start=(k == 0), stop=(k == NCT - 1))
                        pgs = work.tile([128, 512], bf16, tag="pgs", bufs=3,
                                        name=f"r{rep}_pgs{j}_{g}")
                        nc.scalar.copy(pgs[:], pg[:])
                        t_ = work.tile([128, 2, 512], bf16, tag="tcs", bufs=6,
                                       name=f"r{rep}_tcs{j}_{g}")
                        nc.vector.tensor_tensor(
                            t_[:], pgs[:, None, :].broadcast_to([128, 2, 512]),
                            css[:], MUL)
                        tcs_[pr, par] = t_
                    return run

                def combines(pr):
                    def run():
                        te, to = tcs_[pr, 0], tcs_[pr, 1]
                        for a in range(4):  # head within quad
                            l = 4 * (pr % 2) + a
                            r = slice(32 * a, 32 * a + 32)
                            if pr < 2:
                                dst = qTc_[rep, j][:, l // 2, :]
                            else:
                                dst = kT_sb[:, l // 2, c0:c0 + 512]
                            b = 64 * (l % 2)
                            eng = nc.vector if a % 2 == 0 else nc.gpsimd
                            nc.vector.tensor_tensor(
                                dst[b:b + 32], te[r, 0], to[r, 1], SUB)
                            eng.tensor_tensor(
                                dst[b + 32:b + 64], to[r, 0], te[r, 1], ADD)
                    return run

                for pr in range(4):
                    steps.append(parblock(pr, 0))
                    steps.append(parblock(pr, 1))
                    steps.append(combines(pr))

                def v_block(tt):
                    def run():
                        xt = xtn["t"]
                        pv = psum.tile([128, 512], f32, tag="pp", bufs=2,
                                       name=f"r{rep}_pv{j}_{tt}")
                        for k in range(NCT):
                            nc.tensor.matmul(pv[:], xt[:, k, 128 * tt:128 * tt + 128],
                                             wv_sb[:, k, :],
                                             start=(k == 0), stop=(k == NCT - 1))
                        nc.scalar.copy(
                            v_sb[:, 4 * j + tt, :, 0:D],
                            pv[:].rearrange("p (h d) -> p h d", h=HC))
                    return run
                for tt in range(4):
                    steps.append(v_block(tt))
                return steps

            def sweep(rep, j, g):
                """Attention i-loop for head pair (2g, 2g+1) of chunk j."""
                nk = 4 * (j + 1)
                qTc = qTc_[rep, j]
                av = [psum.tile([128, 512], f32, tag="av", bufs=2,
                                name=f"r{rep}_av{j}_{g}_{hh}") for hh in range(2)]
                for i in range(nk):
                    off = 128 * i - 512 * j
                    # bf16 matmuls have no sub-256-column rate penalty, so
                    # trim the causal region at exact 128-block granularity
                    lo = max(off, 0)
                    sps = psum.tile([128, 2, 512], f32, tag="ps", bufs=2,
                                    name=f"r{rep}_sps{j}_{g}_{i}")
                    for hh in range(2):
                        b = 64 * hh
                        nc.tensor.matmul(
                            sps[:, hh, lo:512],
                            kT_sb[b:b + 64, g, 128 * i:128 * i + 128],
                            qTc[b:b + 64, g, lo:512],
                            start=True, stop=True, tile_position=(b, 0))
                    pt = work.tile([128, 2, 512], bf16, tag="pt", bufs=6,
                                   name=f"r{rep}_pt{j}_{g}_{i}")
                    nc.scalar.activation(pt[:, :, lo:512], sps[:, :, lo:512], EXP)
                    if off >= 0:
                        # triangular 0/1 mask on the 128-wide diagonal block
                        nc.vector.tensor_tensor(
                            pt[:, :, off:off + 128], pt[:, :, off:off + 128],
                            mk_sb[:, None, 128:256].broadcast_to([128, 2, 128]),
                            MUL)
                    for hh in range(2):
                        l = 2 * g + hh
                        nc.tensor.matmul(
                            av[hh][:, lo:512],
                            v_sb[:, i, l, :], pt[:, hh, lo:512],
                            start=(i == 0), stop=(i == nk - 1))
                return av

            def normalize(rep, j, g, av, aot):
                for hh in range(2):
                    recip = work.tile([64, 512], f32, tag="recip", bufs=4,
                                      name=f"r{rep}_rc{j}_{g}_{hh}")
                    nc.vector.reciprocal(recip[:], av[hh][D:2 * D, :])
                    nc.vector.tensor_tensor(
                        aot[64 * hh:64 * hh + 64, g, :], av[hh][0:D, :], recip[:], MUL)

            def outproj_steps(rep, j, aot):
                c0 = 512 * j
                steps = []

                def yblock(tt):
                    def run():
                        yst = work.tile([128, 2, 512], f32, tag="yst", bufs=3,
                                        name=f"r{rep}_yst{j}_{tt}")
                        for cc in range(2):
                            yps = psum.tile([128, 512], f32, tag="pp", bufs=2,
                                            name=f"r{rep}_yps{j}_{tt}_{cc}")
                            for ct in range(4):
                                nc.tensor.matmul(
                                    yps[:],
                                    aot[:, ct, 128 * tt:128 * tt + 128],
                                    wout_sb[:, ct, 512 * cc:512 * cc + 512],
                                    start=(ct == 0), stop=(ct == 3))
                            nc.vector.tensor_copy(yst[:, cc], yps[:])
                        nc.sync.dma_start(
                            y_t.ap()[c0 + 128 * tt:c0 + 128 * tt + 128, :],
                            yst[:].rearrange("p a b -> p (a b)"))
                    return run
                for tt in range(4):
                    steps.append(yblock(tt))
                return steps

            # ---- main schedule: per chunk, 4 attention sweeps with the
            # next chunk's projections interleaved at sweep boundaries -----
            for s in make_proj_steps(0, 0):
                s()
            for rep in range(repeat):
                for j in range(NJ):
                    if j + 1 < NJ:
                        nxt = make_proj_steps(rep, j + 1)
                    elif rep + 1 < repeat:
                        nxt = make_proj_steps(rep + 1, 0)
                    else:
                        nxt = []
                    # issue next chunk's DMAs before the first sweep (pure
                    # prefetch, no PE-queue impact), PE-bearing steps at
                    # sweep boundaries
                    for s in nxt[0:1]:
                        s()
                    sched = {0: nxt[1:4], 1: nxt[4:10], 2: nxt[10:14],
                             3: nxt[14:17]}
                    aot = work.tile([128, 4, 512], bf16, tag="aot", bufs=2,
                                    name=f"r{rep}_aot{j}")
                    for g in range(4):
                        av = sweep(rep, j, g)
                        normalize(rep, j, g, av, aot)
                        for s in sched[g]:
                            s()
                    for s in outproj_steps(rep, j, aot):
                        s()

    nc.compile()
    return nc


def _host_inputs(x, W_qkv, W_out):
    """Per-core input dicts (numpy)."""
    import ml_dtypes
    x = np.ascontiguousarray(np.asarray(x), dtype=np.float32)
    W_qkv = np.ascontiguousarray(np.asarray(W_qkv), dtype=np.float32)
    W_out = np.ascontiguousarray(np.asarray(W_out), dtype=np.float32)

    inv_freq = (1.0 / (THETA ** (np.arange(0, D, 2, dtype=np.float32) / D))).astype(np.float32)
    freqs = np.arange(T, dtype=np.float32)[:, None] * inv_freq[None, :]  # [T, 32]
    cs = np.tile(np.cos(freqs).T.astype(np.float32), (4, 1))  # [128, T]
    sn = np.tile(np.sin(freqs).T.astype(np.float32), (4, 1))
    csn = np.ascontiguousarray(np.stack([cs, sn], axis=1)).astype(ml_dtypes.bfloat16)  # [128, 2, T]
    kk = np.arange(128)[:, None]
    cc = np.arange(256)[None, :]
    mk = (cc >= kk + 128).astype(ml_dtypes.bfloat16)  # [128, 256]

    in_maps = []
    for core in range(N_CORES):
        b, hg = core // 2, core % 2
        h0 = HC * hg  # first global head
        # q/k columns permuted into per-quad even/odd projection tiles:
        # group g = s*4 + 2*pair + par holds heads [4*pair, +4), par-parity dims
        cols = []
        for s in range(2):  # 0=q, 1=k
            for pair in range(2):          # head quads
                for par in range(2):       # 0=even-dims tile, 1=odd-dims tile
                    for a in range(4):     # head within quad
                        hglob = h0 + 4 * pair + a
                        for i_ in range(32):
                            cols.append(s * (H * D) + hglob * D + 2 * i_ + par)
        cols = np.asarray(cols)
        wqk = W_qkv[:, cols].copy()
        wqk[:, 0:512] *= np.float32(1.0 / np.sqrt(D))  # fold score scale into Wq
        # [C, 1024] -> [128 part, 8 grp, 8 kt, 128 m]
        wqk = np.ascontiguousarray(
            wqk.reshape(NCT, 128, 8, 128).transpose(1, 2, 0, 3))
        wv = W_qkv[:, 2 * H * D + h0 * D: 2 * H * D + (h0 + HC) * D].copy()
        wout = W_out[h0 * D:(h0 + HC) * D, :].copy()
        in_maps.append({
            "xT": np.ascontiguousarray(x[b].T).astype(ml_dtypes.bfloat16),
            "wqk": wqk.astype(ml_dtypes.bfloat16),
            "wv": wv.astype(ml_dtypes.bfloat16),
            "wout": wout.astype(ml_dtypes.bfloat16),
            "csn": csn, "mk": mk,
        })
    return in_maps


def _get_runtime(repeat=1):
    """Compile once; return a cached sharded jitted callable + metadata."""
    key = ("rt", repeat)
    if key in _CACHE:
        return _CACHE[key]
    import jax
    import numpy as _np
    from jax.sharding import Mesh, PartitionSpec
    from jax.experimental.shard_map import shard_map
    import concourse.mybir as mybir
    from concourse import bass2jax

    nc = _build_program(repeat=repeat)
    bass2jax.install_neuronx_cc_hook()

    partition_name = (nc.partition_id_tensor.name
                      if nc.partition_id_tensor else None)
    in_names, out_names, out_avals, zero_outs = [], [], [], []
    for alloc in nc.m.functions[0].allocations:
        if not isinstance(alloc, mybir.MemoryLocationSet):
            continue
        name = alloc.memorylocations[0].name
        if alloc.kind == "ExternalInput":
            if name != partition_name:
                in_names.append(name)
        elif alloc.kind == "ExternalOutput":
            np_dt = mybir.dt.np(alloc.dtype)
            out_names.append(name)
            out_avals.append(jax.core.ShapedArray(tuple(alloc.tensor_shape), np_dt))
            zero_outs.append(_np.zeros(tuple(alloc.tensor_shape), np_dt))

    n_params = len(in_names)
    n_outs = len(out_names)
    all_in_names = list(in_names) + list(out_names)
    if partition_name is not None:
        all_in_names.append(partition_name)
    donate = tuple(range(n_params, n_params + n_outs))

    def _body(*args):
        operands = list(args)
        if partition_name is not None:
            operands.append(bass2jax.partition_id_tensor())
        outs = bass2jax._bass_exec_p.bind(
            *operands,
            out_avals=tuple(out_avals),
            in_names=tuple(all_in_names),
            out_names=tuple(out_names),
            lowering_input_output_aliases=(),
            sim_require_finite=True,
            sim_require_nnan=True,
            nc=nc,
        )
        return tuple(outs)

    devices = jax.devices()[:N_CORES]
    mesh = Mesh(np.asarray(devices), ("core",))
    in_specs = (PartitionSpec("core"),) * (n_params + n_outs)
    out_specs = (PartitionSpec("core"),) * n_outs
    fn = jax.jit(
        shard_map(_body, mesh=mesh, in_specs=in_specs, out_specs=out_specs,
                  check_rep=False),
        donate_argnums=donate, keep_unused=True)

    rt = dict(fn=fn, in_names=in_names, out_names=out_names,
              zero_outs=zero_outs, mesh=mesh)
    _CACHE[key] = rt
    return rt


def _run(in_maps):
    rt = _get_runtime()
    concat_in = [np.concatenate([np.asarray(in_maps[c][n]) for c in range(N_CORES)],
                                axis=0) for n in rt["in_names"]]
    concat_zeros = [np.zeros((N_CORES * z.shape[0], *z.shape[1:]), z.dtype)
                    for z in rt["zero_outs"]]
    out_arrs = rt["fn"](*concat_in, *concat_zeros)
    y_all = np.asarray(out_arrs[0]).reshape(N_CORES, T, C)
    return y_all


def kernel(x, W_qkv, W_out):
    in_maps = _host_inputs(x, W_qkv, W_out)
    y_all = _run(in_maps)
    y = np.empty((B, T, C), dtype=np.float32)
    for b in range(B):
        y[b] = y_all[2 * b] + y_all[2 * b + 1]
    return y



# revision 10
# speedup vs baseline: 1.0273x; 1.0273x over previous
"""Multi-head causal self-attention with RoPE for Trainium2 (8 NeuronCores).

Problem: B=4, T=2048, C=1024, H=16 heads, D=64, fused QKV + causal softmax
attention + out-projection, fp32 I/O.

Sharding (Megatron-style): core c -> batch b = c//2, heads [8*(c%2), +8).
Each core computes its 8 heads' attention for its batch and a row-parallel
partial of the out-projection; the host sums the two partials per batch.

Per-core design (v2 — ACT-paced software pipeline):
  - All matmuls in bf16. q/k head-major: head h owns 64 contiguous
    partitions -> scores are one K=64 matmul per head; the two heads of a
    group run CONCURRENTLY in the PE array via row tile_position (0/64).
  - The sweep over (g, i) slots is software-pipelined one slot ahead:
    scores+exp of slot s+1 are issued before attn*V of slot s, so the
    Activation engine (the critical engine: one ~1us exp per slot) never
    waits on the PE queue.
  - All non-exp work is kept OFF the ACT engine: PSUM->SBUF copies go to
    GPSIMD (pool), rope-combine/normalize/mask stay on DVE, y staging
    alternates DVE/pool.
  - Projections for chunk j+1/j+2 and the out-projection of chunk j-1 are
    queued as "filler" steps and drained into the per-slot PE slack
    (exp_time - scores - av) via a credit scheduler, with forced pacing so
    each chunk's prerequisites finish before its sweeps begin.
  - v carries 64 replicated ones-columns: attn*V lands the softmax
    denominators pre-broadcast on partitions 64-127.
  - PSUM: scores 2x2-bank slots + 2 banks attn*V + 2 banks projections.
"""

import os
import numpy as np

# bisect toggles (dev only; default = full v2 behavior)
_VAR = os.environ.get("KERNEL_VARIANT", "full")

B, T, C = 4, 2048, 1024
H, D = 16, 64
HC = 8               # heads per core
N_CORES = 8
THETA = 10000.0
NJ = T // 512        # 4 query/column chunks
NCT = C // 128       # 8 contraction tiles for projections

_CACHE = {}


def _build_program(repeat=1):
    """Build the per-core program. repeat>1 replays the whole computation
    (same inputs/outputs) for clean wall-clock timing."""
    import contextlib
    from collections import deque
    import concourse.tile as tile
    import concourse.mybir as mybir
    from concourse import bacc

    f32 = mybir.dt.float32
    bf16 = mybir.dt.bfloat16
    EXP = mybir.ActivationFunctionType.Exp
    MUL = mybir.AluOpType.mult
    SUB = mybir.AluOpType.subtract
    ADD = mybir.AluOpType.add

    nc = bacc.Bacc("TRN2", target_bir_lowering=False, debug=False)
    xT_t = nc.dram_tensor("xT", [C, T], bf16, kind="ExternalInput")
    wqk_t = nc.dram_tensor("wqk", [128, 8, NCT, 128], bf16, kind="ExternalInput")
    wv_t = nc.dram_tensor("wv", [C, HC * D], bf16, kind="ExternalInput")
    wout_t = nc.dram_tensor("wout", [HC * D, C], bf16, kind="ExternalInput")
    csn_t = nc.dram_tensor("csn", [128, 2, T], bf16, kind="ExternalInput")
    mk_t = nc.dram_tensor("mk", [128, 256], bf16, kind="ExternalInput")
    y_t = nc.dram_tensor("y", [T, C], f32, kind="ExternalOutput")

    with tile.TileContext(nc) as tc:
        with contextlib.ExitStack() as ctx:
            singles = ctx.enter_context(tc.tile_pool(name="singles", bufs=1))
            psum = ctx.enter_context(tc.tile_pool(name="psum", bufs=1, space="PSUM"))
            work = ctx.enter_context(tc.tile_pool(name="work", bufs=1))

            # ---- resident tensors -------------------------------------------
            kT_sb = singles.tile([128, 4, T], bf16, name="kT_sb")
            v_sb = singles.tile([128, T // 128, HC, 2 * D], bf16, name="v_sb")
            wqk_sb = singles.tile([128, 8, NCT, 128], bf16, name="wqk_sb")
            wv_sb = singles.tile([128, NCT, HC * D], bf16, name="wv_sb")
            wout_sb = singles.tile([128, 4, C], bf16, name="wout_sb")
            mk_sb = singles.tile([128, 256], bf16, name="mk_sb")

            # 64 replicated ones-columns: AV lands softmax denominators
            # pre-broadcast on partitions 64-127
            nc.gpsimd.memset(v_sb[:, :, :, D:2 * D], 1.0)
            for g2 in range(4):
                nc.sync.dma_start(wqk_sb[:, 2 * g2:2 * g2 + 2],
                                  wqk_t.ap()[:, 2 * g2:2 * g2 + 2])

            # per-chunk state
            qTc_ = {}
            css_ = {}
            xtn_ = {}
            av_ = {}

            def make_proj_steps(rep, j):
                """Filler steps for chunk (rep, j)'s qkv projections + rope.
                Ordered so g=0/1 sweep prerequisites complete first."""
                c0 = 512 * j
                tcs_ = {}

                def dmas():
                    css = work.tile([128, 2, 512], bf16, tag="csn", bufs=3,
                                    name=f"r{rep}_csn{j}")
                    nc.sync.dma_start(css[:], csn_t.ap()[:, :, c0:c0 + 512])
                    css_[rep, j] = css
                    xt = work.tile([128, NCT, 512], bf16, tag="xtn", bufs=3,
                                   name=f"r{rep}_xtn{j}")
                    src = xT_t.ap().rearrange("(kt p) n -> p kt n", p=128)
                    nc.sync.dma_start(xt[:, 0:4], src[:, 0:4, c0:c0 + 512])
                    nc.sync.dma_start(xt[:, 4:8], src[:, 4:8, c0:c0 + 512])
                    xtn_[rep, j] = xt
                    qTc_[rep, j] = work.tile([128, 4, 512], bf16, tag="qTc",
                                             bufs=3, name=f"r{rep}_qTc{j}")

                def parblock(pr, par):
                    # one 128-dim projection tile (even or odd dims of a quad)
                    def run():
                        css = css_[rep, j]
                        xt = xtn_[rep, j]
                        g = 2 * pr + par
                        pg = psum.tile([128, 512], f32, tag="pp", bufs=2,
                                       name=f"r{rep}_pg{j}_{g}")
                        for k in range(NCT):
                            nc.tensor.matmul(pg[:], wqk_sb[:, g, k, :], xt[:, k],
                                             start=(k == 0), stop=(k == NCT - 1))
                        t_ = work.tile([128, 2, 512], bf16, tag="tcs", bufs=6,
                                       name=f"r{rep}_tcs{j}_{g}")
                        if _VAR in ("actcopy", "dvecopy"):
                            pgs = work.tile([128, 512], bf16, tag="pgs", bufs=3,
                                            name=f"r{rep}_pgs{j}_{g}")
                            eng = nc.scalar if _VAR == "actcopy" else nc.vector
                            if _VAR == "actcopy":
                                nc.scalar.copy(pgs[:], pg[:])
                            else:
                                nc.vector.tensor_copy(pgs[:], pg[:])
                            nc.vector.tensor_tensor(
                                t_[:], pgs[:, None, :].broadcast_to([128, 2, 512]),
                                css[:], MUL)
                        else:
                            nc.vector.tensor_tensor(
                                t_[:], pg[:, None, :].broadcast_to([128, 2, 512]),
                                css[:], MUL)
                        tcs_[pr, par] = t_
                    return run

                def combines(pr):
                    def run():
                        te, to = tcs_[pr, 0], tcs_[pr, 1]
                        for a in range(4):  # head within quad
                            l = 4 * (pr % 2) + a
                            r = slice(32 * a, 32 * a + 32)
                            if pr < 2:
                                dst = qTc_[rep, j][:, l // 2, :]
                            else:
                                dst = kT_sb[:, l // 2, c0:c0 + 512]
                            b = 64 * (l % 2)
                            nc.vector.tensor_tensor(
                                dst[b:b + 32], te[r, 0], to[r, 1], SUB)
                            nc.gpsimd.tensor_tensor(
                                dst[b + 32:b + 64], to[r, 0], te[r, 1], ADD)
                    return run

                def v_block(tt):
                    def run():
                        xt = xtn_[rep, j]
                        pv = psum.tile([128, 512], f32, tag="pp", bufs=2,
                                       name=f"r{rep}_pv{j}_{tt}")
                        for k in range(NCT):
                            nc.tensor.matmul(pv[:], xt[:, k, 128 * tt:128 * tt + 128],
                                             wv_sb[:, k, :],
                                             start=(k == 0), stop=(k == NCT - 1))
                        nc.vector.tensor_copy(
                            v_sb[:, 4 * j + tt, :, 0:D],
                            pv[:].rearrange("p (h d) -> p h d", h=HC))
                    return run

                return [("dma", dmas),
                        ("pb", parblock(0, 0)), ("pb", parblock(0, 1)),
                        ("cb", combines(0)),
                        ("pb", parblock(2, 0)), ("pb", parblock(2, 1)),
                        ("cb", combines(2)),
                        ("vb", v_block(0)),
                        ("pb", parblock(1, 0)), ("pb", parblock(1, 1)),
                        ("cb", combines(1)),
                        ("pb", parblock(3, 0)), ("pb", parblock(3, 1)),
                        ("cb", combines(3)),
                        ("vb", v_block(1)), ("vb", v_block(2)),
                        ("vb", v_block(3))]

            def make_outproj_steps(rep, j, aot):
                c0 = 512 * j

                def yblock(tt):
                    def run():
                        yst = work.tile([128, 2, 512], f32, tag="yst", bufs=3,
                                        name=f"r{rep}_yst{j}_{tt}")
                        for cc in range(2):
                            yps = psum.tile([128, 512], f32, tag="pp", bufs=2,
                                            name=f"r{rep}_yps{j}_{tt}_{cc}")
                            for ct in range(4):
                                nc.tensor.matmul(
                                    yps[:],
                                    aot[:, ct, 128 * tt:128 * tt + 128],
                                    wout_sb[:, ct, 512 * cc:512 * cc + 512],
                                    start=(ct == 0), stop=(ct == 3))
                            nc.vector.tensor_copy(yst[:, cc], yps[:])
                        nc.sync.dma_start(
                            y_t.ap()[c0 + 128 * tt:c0 + 128 * tt + 128, :],
                            yst[:].rearrange("p a b -> p (a b)"))
                    return run
                return [("yb", yblock(tt)) for tt in range(4)]

            def scores_step(rep, j, g, i):
                """Scores pair + exp (+ causal mask) for slot (g, i)."""
                off = 128 * i - 512 * j
                lo = max(off, 0)
                qTc = qTc_[rep, j]
                sps = psum.tile([128, 2, 512], f32, tag="ps", bufs=2,
                                name=f"r{rep}_sps{j}_{g}_{i}")
                for hh in range(2):
                    b = 64 * hh
                    nc.tensor.matmul(
                        sps[:, hh, lo:512],
                        kT_sb[b:b + 64, g, 128 * i:128 * i + 128],
                        qTc[b:b + 64, g, lo:512],
                        start=True, stop=True, tile_position=(b, 0))
                pt = work.tile([128, 2, 512], bf16, tag="pt", bufs=6,
                               name=f"r{rep}_pt{j}_{g}_{i}")
                nc.scalar.activation(pt[:, :, lo:512], sps[:, :, lo:512], EXP)
                if off >= 0:
                    # triangular 0/1 mask on the 128-wide diagonal block
                    nc.vector.tensor_tensor(
                        pt[:, :, off:off + 128], pt[:, :, off:off + 128],
                        mk_sb[:, None, 128:256].broadcast_to([128, 2, 128]),
                        MUL)
                return pt, lo

            def av_step(rep, j, g, i, pt, lo):
                nk = 4 * (j + 1)
                if i == 0:
                    av_[g] = [psum.tile([128, 512], f32, tag="av", bufs=2,
                                        name=f"r{rep}_av{j}_{g}_{hh}")
                              for hh in range(2)]
                for hh in range(2):
                    l = 2 * g + hh
                    nc.tensor.matmul(
                        av_[g][hh][:, lo:512],
                        v_sb[:, i, l, :], pt[:, hh, lo:512],
                        start=(i == 0), stop=(i == nk - 1))

            def normalize(rep, j, g, aot):
                for hh in range(2):
                    recip = work.tile([64, 512], f32, tag="recip", bufs=4,
                                      name=f"r{rep}_rc{j}_{g}_{hh}")
                    nc.vector.reciprocal(recip[:], av_[g][hh][D:2 * D, :])
                    nc.vector.tensor_tensor(
                        aot[64 * hh:64 * hh + 64, g, :], av_[g][hh][0:D, :],
                        recip[:], MUL)

            # ---- filler queue + credit scheduler ----------------------------
            COST = {"dma": 0.0, "pb": 1300.0, "cb": 100.0, "vb": 1400.0,
                    "yb": 1500.0}
            CAP = 2600.0
            queue = deque()   # (due_ci, cost, fn)
            credit = [0.0]

            def pop_run():
                due, cost, fn = queue.popleft()
                fn()
                return cost

            def qcost_due(limit):
                return sum(c for (d, c, _) in queue if d <= limit)

            CH = [(r, j) for r in range(repeat) for j in range(NJ)]

            # ---- startup: chunk 0 projections inline ------------------------
            steps0 = make_proj_steps(0, 0)
            steps0[0][1]()                       # chunk-0 dmas first
            nc.sync.dma_start(mk_sb[:], mk_t.ap())
            nc.sync.dma_start(wv_sb[:],
                              wv_t.ap().rearrange("(kt p) n -> p kt n", p=128))
            for kind, fn in steps0[1:]:
                fn()
            nc.sync.dma_start(wout_sb[:],
                              wout_t.ap().rearrange("(ct p) n -> p ct n", p=128))
            if len(CH) > 1:
                for kind, fn in make_proj_steps(*CH[1]):
                    queue.append((1, COST[kind], fn))

            # ---- main schedule ----------------------------------------------
            for ci, (rep, j) in enumerate(CH):
                if ci + 2 < len(CH):
                    for kind, fn in make_proj_steps(*CH[ci + 2]):
                        queue.append((ci + 2, COST[kind], fn))
                while queue and queue[0][0] <= ci:
                    pop_run()

                nk = 4 * (j + 1)
                slots = [(g, i) for g in range(4) for i in range(nk)]
                aot = work.tile([128, 4, 512], bf16, tag="aot", bufs=3,
                                name=f"r{rep}_aot{j}")
                if _VAR == "nopipe":
                    pend = None
                else:
                    pend = scores_step(rep, j, *slots[0])
                for s in range(len(slots)):
                    g, i = slots[s]
                    if _VAR == "nopipe":
                        pt, lo = scores_step(rep, j, g, i)
                    else:
                        pt, lo = pend
                        if s + 1 < len(slots):
                            pend = scores_step(rep, j, *slots[s + 1])
                    av_step(rep, j, g, i, pt, lo)
                    if i == nk - 1:
                        normalize(rep, j, g, aot)
                    # fillers in the exp shadow
                    w = 512 - lo
                    credit[0] = min(credit[0] + 0.9 * w + 85.0, CAP)
                    slots_left = len(slots) - s
                    budget_spent = 0.0
                    while queue:
                        forced = (qcost_due(ci + 1) >
                                  (slots_left - 1) * 500.0)
                        if credit[0] >= queue[0][1] or forced:
                            c = pop_run()
                            credit[0] -= c
                            budget_spent += c
                            if budget_spent > 3000.0 and not forced:
                                break
                        else:
                            break
                for kind, fn in make_outproj_steps(rep, j, aot):
                    queue.append((ci + 2, COST[kind], fn))

            while queue:
                pop_run()

    nc.compile()
    return nc


def _host_inputs(x, W_qkv, W_out):
    """Per-core input dicts (numpy)."""
    import ml_dtypes
    x = np.ascontiguousarray(np.asarray(x), dtype=np.float32)
    W_qkv = np.ascontiguousarray(np.asarray(W_qkv), dtype=np.float32)
    W_out = np.ascontiguousarray(np.asarray(W_out), dtype=np.float32)

    inv_freq = (1.0 / (THETA ** (np.arange(0, D, 2, dtype=np.float32) / D))).astype(np.float32)
    freqs = np.arange(T, dtype=np.float32)[:, None] * inv_freq[None, :]  # [T, 32]
    cs = np.tile(np.cos(freqs).T.astype(np.float32), (4, 1))  # [128, T]
    sn = np.tile(np.sin(freqs).T.astype(np.float32), (4, 1))
    csn = np.ascontiguousarray(np.stack([cs, sn], axis=1)).astype(ml_dtypes.bfloat16)  # [128, 2, T]
    kk = np.arange(128)[:, None]
    cc = np.arange(256)[None, :]
    mk = (cc >= kk + 128).astype(ml_dtypes.bfloat16)  # [128, 256]

    in_maps = []
    for core in range(N_CORES):
        b, hg = core // 2, core % 2
        h0 = HC * hg  # first global head
        # q/k columns permuted into per-quad even/odd projection tiles:
        # group g = s*4 + 2*pair + par holds heads [4*pair, +4), par-parity dims
        cols = []
        for s in range(2):  # 0=q, 1=k
            for pair in range(2):          # head quads
                for par in range(2):       # 0=even-dims tile, 1=odd-dims tile
                    for a in range(4):     # head within quad
                        hglob = h0 + 4 * pair + a
                        for i_ in range(32):
                            cols.append(s * (H * D) + hglob * D + 2 * i_ + par)
        cols = np.asarray(cols)
        wqk = W_qkv[:, cols].copy()
        wqk[:, 0:512] *= np.float32(1.0 / np.sqrt(D))  # fold score scale into Wq
        # [C, 1024] -> [128 part, 8 grp, 8 kt, 128 m]
        wqk = np.ascontiguousarray(
            wqk.reshape(NCT, 128, 8, 128).transpose(1, 2, 0, 3))
        wv = W_qkv[:, 2 * H * D + h0 * D: 2 * H * D + (h0 + HC) * D].copy()
        wout = W_out[h0 * D:(h0 + HC) * D, :].copy()
        in_maps.append({
            "xT": np.ascontiguousarray(x[b].T).astype(ml_dtypes.bfloat16),
            "wqk": wqk.astype(ml_dtypes.bfloat16),
            "wv": wv.astype(ml_dtypes.bfloat16),
            "wout": wout.astype(ml_dtypes.bfloat16),
            "csn": csn, "mk": mk,
        })
    return in_maps


def _get_runtime(repeat=1):
    """Compile once; return a cached sharded jitted callable + metadata."""
    key = ("rt", repeat)
    if key in _CACHE:
        return _CACHE[key]
    import jax
    import numpy as _np
    from jax.sharding import Mesh, PartitionSpec
    from jax.experimental.shard_map import shard_map
    import concourse.mybir as mybir
    from concourse import bass2jax

    nc = _build_program(repeat=repeat)
    bass2jax.install_neuronx_cc_hook()

    partition_name = (nc.partition_id_tensor.name
                      if nc.partition_id_tensor else None)
    in_names, out_names, out_avals, zero_outs = [], [], [], []
    for alloc in nc.m.functions[0].allocations:
        if not isinstance(alloc, mybir.MemoryLocationSet):
            continue
        name = alloc.memorylocations[0].name
        if alloc.kind == "ExternalInput":
            if name != partition_name:
                in_names.append(name)
        elif alloc.kind == "ExternalOutput":
            np_dt = mybir.dt.np(alloc.dtype)
            out_names.append(name)
            out_avals.append(jax.core.ShapedArray(tuple(alloc.tensor_shape), np_dt))
            zero_outs.append(_np.zeros(tuple(alloc.tensor_shape), np_dt))

    n_params = len(in_names)
    n_outs = len(out_names)
    all_in_names = list(in_names) + list(out_names)
    if partition_name is not None:
        all_in_names.append(partition_name)
    donate = tuple(range(n_params, n_params + n_outs))

    def _body(*args):
        operands = list(args)
        if partition_name is not None:
            operands.append(bass2jax.partition_id_tensor())
        outs = bass2jax._bass_exec_p.bind(
            *operands,
            out_avals=tuple(out_avals),
            in_names=tuple(all_in_names),
            out_names=tuple(out_names),
            lowering_input_output_aliases=(),
            sim_require_finite=True,
            sim_require_nnan=True,
            nc=nc,
        )
        return tuple(outs)

    devices = jax.devices()[:N_CORES]
    mesh = Mesh(np.asarray(devices), ("core",))
    in_specs = (PartitionSpec("core"),) * (n_params + n_outs)
    out_specs = (PartitionSpec("core"),) * n_outs
    fn = jax.jit(
        shard_map(_body, mesh=mesh, in_specs=in_specs, out_specs=out_specs,
                  check_rep=False),
        donate_argnums=donate, keep_unused=True)

    rt = dict(fn=fn, in_names=in_names, out_names=out_names,
              zero_outs=zero_outs, mesh=mesh)
    _CACHE[key] = rt
    return rt


def _run(in_maps):
    rt = _get_runtime()
    concat_in = [np.concatenate([np.asarray(in_maps[c][n]) for c in range(N_CORES)],
                                axis=0) for n in rt["in_names"]]
    concat_zeros = [np.zeros((N_CORES * z.shape[0], *z.shape[1:]), z.dtype)
                    for z in rt["zero_outs"]]
    out_arrs = rt["fn"](*concat_in, *concat_zeros)
    y_all = np.asarray(out_arrs[0]).reshape(N_CORES, T, C)
    return y_all


def kernel(x, W_qkv, W_out):
    in_maps = _host_inputs(x, W_qkv, W_out)
    y_all = _run(in_maps)
    y = np.empty((B, T, C), dtype=np.float32)
    for b in range(B):
        y[b] = y_all[2 * b] + y_all[2 * b + 1]
    return y


# revision 11
# speedup vs baseline: 1.1927x; 1.1609x over previous
"""Multi-head causal self-attention with RoPE for Trainium2 (8 NeuronCores).

Problem: B=4, T=2048, C=1024, H=16 heads, D=64, fused QKV + causal softmax
attention + out-projection, fp32 I/O.

Sharding (Megatron-style): core c -> batch b = c//2, heads [8*(c%2), +8).
Each core computes its 8 heads' attention for its batch and a row-parallel
partial of the out-projection; the host sums the two partials per batch.

Per-core design:
  - All matmuls in bf16 (full PE rate; inputs quantized host-side).
  - q/k stored head-major: head h owns 64 contiguous partitions
    [32 even-rot | 32 odd-rot] -> scores are ONE K=64 matmul per head.
    RoPE projects through separate even/odd-dim tiles so the rotation
    combines read equal partition bases (HW SB+SB constraint) and write
    shifted output partitions.
  - W_qkv/W_v/W_out resident in SBUF (loaded once).
  - causal trim at exact 128-block granularity (bf16 matmuls have no
    narrow-free-dim rate penalty).
  - v carries 64 replicated ones-columns: attention*V lands the softmax
    denominators pre-broadcast on partitions 64-127 (no partition
    broadcast on the normalize path).
  - PSUM: scores 2x2-bank slots + 2 banks attn*V + 2 banks projections.
  - chunk j+1 projections are emitted interleaved at chunk j's sweep
    boundaries; reps pipeline across the boundary for repeat>1 timing.
"""

import numpy as np

B, T, C = 4, 2048, 1024
H, D = 16, 64
HC = 8               # heads per core
N_CORES = 8
THETA = 10000.0
NJ = T // 512        # 4 query/column chunks
NCT = C // 128       # 8 contraction tiles for projections

_CACHE = {}


def _build_program(repeat=1):
    """Build the per-core program. repeat>1 replays the whole computation
    (same inputs/outputs) for clean wall-clock timing."""
    import contextlib
    import concourse.tile as tile
    import concourse.mybir as mybir
    from concourse import bacc

    f32 = mybir.dt.float32
    f32r = mybir.dt.float32r
    bf16 = mybir.dt.bfloat16
    EXP = mybir.ActivationFunctionType.Exp
    MUL = mybir.AluOpType.mult
    SUB = mybir.AluOpType.subtract
    ADD = mybir.AluOpType.add

    nc = bacc.Bacc("TRN2", target_bir_lowering=False, debug=False)
    xT_t = nc.dram_tensor("xT", [C, T], bf16, kind="ExternalInput")
    wqk_t = nc.dram_tensor("wqk", [128, 8, NCT, 128], bf16, kind="ExternalInput")
    wv_t = nc.dram_tensor("wv", [C, HC * D], bf16, kind="ExternalInput")
    wout_t = nc.dram_tensor("wout", [HC * D, C], bf16, kind="ExternalInput")
    csn_t = nc.dram_tensor("csn", [128, 2, T], bf16, kind="ExternalInput")
    mk_t = nc.dram_tensor("mk", [128, 256], bf16, kind="ExternalInput")
    y_t = nc.dram_tensor("y", [T, C], f32, kind="ExternalOutput")

    with tile.TileContext(nc) as tc:
        with contextlib.ExitStack() as ctx:
            singles = ctx.enter_context(tc.tile_pool(name="singles", bufs=1))
            psum = ctx.enter_context(tc.tile_pool(name="psum", bufs=1, space="PSUM"))
            work = ctx.enter_context(tc.tile_pool(name="work", bufs=1))

            # ---- resident tensors -------------------------------------------
            kT_sb = singles.tile([128, 4, T], bf16, name="kT_sb")
            v_sb = singles.tile([128, T // 128, HC, 2 * D], bf16, name="v_sb")
            wqk_sb = singles.tile([128, 8, NCT, 128], bf16, name="wqk_sb")
            wv_sb = singles.tile([128, NCT, HC * D], bf16, name="wv_sb")
            wout_sb = singles.tile([128, 4, C], bf16, name="wout_sb")
            mk_sb = singles.tile([128, 256], bf16, name="mk_sb")

            # 64 replicated ones-columns: AV lands softmax denominators
            # pre-broadcast on partitions 64-127 (output partitions are free)
            nc.gpsimd.memset(v_sb[:, :, :, D:2 * D], 1.0)
            for g2 in range(4):
                nc.sync.dma_start(wqk_sb[:, 2 * g2:2 * g2 + 2],
                                  wqk_t.ap()[:, 2 * g2:2 * g2 + 2])
            nc.sync.dma_start(mk_sb[:], mk_t.ap())
            nc.sync.dma_start(wv_sb[:],
                              wv_t.ap().rearrange("(kt p) n -> p kt n", p=128))
            nc.sync.dma_start(wout_sb[:],
                              wout_t.ap().rearrange("(ct p) n -> p ct n", p=128))

            # per-chunk state (created by the proj steps, used by phase 2);
            # keyed (rep, j) so projections pipeline across rep boundaries
            qTc_ = {}
            css_ = {}

            def make_proj_steps(rep, j):
                """Micro-step closures for chunk (rep, j)'s projections+rope:
                [dmas] + per head-quad [even-proj, odd-proj, combines] + v."""
                c0 = 512 * j
                xtn = {}
                tcs_ = {}
                steps = []

                def dmas():
                    css = work.tile([128, 2, 512], bf16, tag="csn", bufs=3,
                                    name=f"r{rep}_csn{j}")
                    nc.sync.dma_start(css[:], csn_t.ap()[:, :, c0:c0 + 512])
                    css_[rep, j] = css
                    xt = work.tile([128, NCT, 512], bf16, tag="xtn", bufs=3,
                                   name=f"r{rep}_xtn{j}")
                    src = xT_t.ap().rearrange("(kt p) n -> p kt n", p=128)
                    nc.sync.dma_start(xt[:, 0:4], src[:, 0:4, c0:c0 + 512])
                    nc.sync.dma_start(xt[:, 4:8], src[:, 4:8, c0:c0 + 512])
                    xtn["t"] = xt
                    qTc_[rep, j] = work.tile([128, 4, 512], bf16, tag="qTc", bufs=2,
                                             name=f"r{rep}_qTc{j}")
                steps.append(dmas)

                def parblock(pr, par):
                    # one 128-dim projection tile (even or odd dims of a quad)
                    def run():
                        css = css_[rep, j]
                        xt = xtn["t"]
                        g = 2 * pr + par
                        pg = psum.tile([128, 512], f32, tag="pp", bufs=2,
                                       name=f"r{rep}_pg{j}_{g}")
                        for k in range(NCT):
                            nc.tensor.matmul(pg[:], wqk_sb[:, g, k, :], xt[:, k],
                                             start=(k == 0), stop=(k == NCT - 1))
                        pgs = work.tile([128, 512], bf16, tag="pgs", bufs=3,
                                        name=f"r{rep}_pgs{j}_{g}")
                        nc.scalar.copy(pgs[:], pg[:])
                        t_ = work.tile([128, 2, 512], bf16, tag="tcs", bufs=6,
                                       name=f"r{rep}_tcs{j}_{g}")
                        nc.vector.tensor_tensor(
                            t_[:], pgs[:, None, :].broadcast_to([128, 2, 512]),
                            css[:], MUL)
                        tcs_[pr, par] = t_
                    return run

                def combines(pr):
                    def run():
                        te, to = tcs_[pr, 0], tcs_[pr, 1]
                        for a in range(4):  # head within quad
                            l = 4 * (pr % 2) + a
                            r = slice(32 * a, 32 * a + 32)
                            if pr < 2:
                                dst = qTc_[rep, j][:, l // 2, :]
                            else:
                                dst = kT_sb[:, l // 2, c0:c0 + 512]
                            b = 64 * (l % 2)
                            eng = nc.vector if a % 2 == 0 else nc.gpsimd
                            nc.vector.tensor_tensor(
                                dst[b:b + 32], te[r, 0], to[r, 1], SUB)
                            eng.tensor_tensor(
                                dst[b + 32:b + 64], to[r, 0], te[r, 1], ADD)
                    return run

                for pr in range(4):
                    steps.append(parblock(pr, 0))
                    steps.append(parblock(pr, 1))
                    steps.append(combines(pr))

                def v_block(tt):
                    def run():
                        xt = xtn["t"]
                        pv = psum.tile([128, 512], f32, tag="pp", bufs=2,
                                       name=f"r{rep}_pv{j}_{tt}")
                        for k in range(NCT):
                            nc.tensor.matmul(pv[:], xt[:, k, 128 * tt:128 * tt + 128],
                                             wv_sb[:, k, :],
                                             start=(k == 0), stop=(k == NCT - 1))
                        nc.scalar.copy(
                            v_sb[:, 4 * j + tt, :, 0:D],
                            pv[:].rearrange("p (h d) -> p h d", h=HC))
                    return run
                for tt in range(4):
                    steps.append(v_block(tt))
                return steps

            def sweep(rep, j, g):
                """Attention i-loop for head pair (2g, 2g+1) of chunk j."""
                nk = 4 * (j + 1)
                qTc = qTc_[rep, j]
                av = [psum.tile([128, 512], f32, tag="av", bufs=2,
                                name=f"r{rep}_av{j}_{g}_{hh}") for hh in range(2)]
                for i in range(nk):
                    off = 128 * i - 512 * j
                    # bf16 matmuls have no sub-256-column rate penalty, so
                    # trim the causal region at exact 128-block granularity
                    lo = max(off, 0)
                    sps = psum.tile([128, 2, 512], f32, tag="ps", bufs=2,
                                    name=f"r{rep}_sps{j}_{g}_{i}")
                    for hh in range(2):
                        b = 64 * hh
                        nc.tensor.matmul(
                            sps[:, hh, lo:512],
                            kT_sb[b:b + 64, g, 128 * i:128 * i + 128],
                            qTc[b:b + 64, g, lo:512],
                            start=True, stop=True, tile_position=(b, 0))
                    pt = work.tile([128, 2, 512], bf16, tag="pt", bufs=6,
                                   name=f"r{rep}_pt{j}_{g}_{i}")
                    nc.scalar.activation(pt[:, :, lo:512], sps[:, :, lo:512], EXP)
                    if off >= 0:
                        # triangular 0/1 mask on the 128-wide diagonal block
                        nc.vector.tensor_tensor(
                            pt[:, :, off:off + 128], pt[:, :, off:off + 128],
                            mk_sb[:, None, 128:256].broadcast_to([128, 2, 128]),
                            MUL)
                    for hh in range(2):
                        l = 2 * g + hh
                        nc.tensor.matmul(
                            av[hh][:, lo:512],
                            v_sb[:, i, l, :], pt[:, hh, lo:512],
                            start=(i == 0), stop=(i == nk - 1))
                return av

            def normalize(rep, j, g, av, aot):
                for hh in range(2):
                    recip = work.tile([64, 512], f32, tag="recip", bufs=4,
                                      name=f"r{rep}_rc{j}_{g}_{hh}")
                    nc.vector.reciprocal(recip[:], av[hh][D:2 * D, :])
                    nc.vector.tensor_tensor(
                        aot[64 * hh:64 * hh + 64, g, :], av[hh][0:D, :], recip[:], MUL)

            def outproj_steps(rep, j, aot):
                c0 = 512 * j
                steps = []

                def yblock(tt):
                    def run():
                        yst = work.tile([128, 2, 512], f32, tag="yst", bufs=3,
                                        name=f"r{rep}_yst{j}_{tt}")
                        for cc in range(2):
                            yps = psum.tile([128, 512], f32, tag="pp", bufs=2,
                                            name=f"r{rep}_yps{j}_{tt}_{cc}")
                            for ct in range(4):
                                nc.tensor.matmul(
                                    yps[:],
                                    aot[:, ct, 128 * tt:128 * tt + 128],
                                    wout_sb[:, ct, 512 * cc:512 * cc + 512],
                                    start=(ct == 0), stop=(ct == 3))
                            nc.vector.tensor_copy(yst[:, cc], yps[:])
                        nc.sync.dma_start(
                            y_t.ap()[c0 + 128 * tt:c0 + 128 * tt + 128, :],
                            yst[:].rearrange("p a b -> p (a b)"))
                    return run
                for tt in range(4):
                    steps.append(yblock(tt))
                return steps

            # ---- main schedule: per chunk, 4 attention sweeps with the
            # next chunk's projections interleaved at sweep boundaries -----
            for s in make_proj_steps(0, 0):
                s()
            for rep in range(repeat):
                for j in range(NJ):
                    if j + 1 < NJ:
                        nxt = make_proj_steps(rep, j + 1)
                    elif rep + 1 < repeat:
                        nxt = make_proj_steps(rep + 1, 0)
                    else:
                        nxt = []
                    # issue next chunk's DMAs before the first sweep (pure
                    # prefetch, no PE-queue impact), PE-bearing steps at
                    # sweep boundaries
                    for s in nxt[0:1]:
                        s()
                    sched = {0: nxt[1:4], 1: nxt[4:10], 2: nxt[10:14],
                             3: nxt[14:17]}
                    aot = work.tile([128, 4, 512], bf16, tag="aot", bufs=2,
                                    name=f"r{rep}_aot{j}")
                    for g in range(4):
                        av = sweep(rep, j, g)
                        normalize(rep, j, g, av, aot)
                        for s in sched[g]:
                            s()
                    for s in outproj_steps(rep, j, aot):
                        s()

    nc.compile()
    return nc


def _host_inputs(x, W_qkv, W_out):
    """Per-core input dicts (numpy)."""
    import ml_dtypes
    x = np.ascontiguousarray(np.asarray(x), dtype=np.float32)
    W_qkv = np.ascontiguousarray(np.asarray(W_qkv), dtype=np.float32)
    W_out = np.ascontiguousarray(np.asarray(W_out), dtype=np.float32)

    inv_freq = (1.0 / (THETA ** (np.arange(0, D, 2, dtype=np.float32) / D))).astype(np.float32)
    freqs = np.arange(T, dtype=np.float32)[:, None] * inv_freq[None, :]  # [T, 32]
    cs = np.tile(np.cos(freqs).T.astype(np.float32), (4, 1))  # [128, T]
    sn = np.tile(np.sin(freqs).T.astype(np.float32), (4, 1))
    csn = np.ascontiguousarray(np.stack([cs, sn], axis=1)).astype(ml_dtypes.bfloat16)  # [128, 2, T]
    kk = np.arange(128)[:, None]
    cc = np.arange(256)[None, :]
    mk = (cc >= kk + 128).astype(ml_dtypes.bfloat16)  # [128, 256]

    in_maps = []
    for core in range(N_CORES):
        b, hg = core // 2, core % 2
        h0 = HC * hg  # first global head
        # q/k columns permuted into per-quad even/odd projection tiles:
        # group g = s*4 + 2*pair + par holds heads [4*pair, +4), par-parity dims
        cols = []
        for s in range(2):  # 0=q, 1=k
            for pair in range(2):          # head quads
                for par in range(2):       # 0=even-dims tile, 1=odd-dims tile
                    for a in range(4):     # head within quad
                        hglob = h0 + 4 * pair + a
                        for i_ in range(32):
                            cols.append(s * (H * D) + hglob * D + 2 * i_ + par)
        cols = np.asarray(cols)
        wqk = W_qkv[:, cols].copy()
        wqk[:, 0:512] *= np.float32(1.0 / np.sqrt(D))  # fold score scale into Wq
        # [C, 1024] -> [128 part, 8 grp, 8 kt, 128 m]
        wqk = np.ascontiguousarray(
            wqk.reshape(NCT, 128, 8, 128).transpose(1, 2, 0, 3))
        wv = W_qkv[:, 2 * H * D + h0 * D: 2 * H * D + (h0 + HC) * D].copy()
        wout = W_out[h0 * D:(h0 + HC) * D, :].copy()
        in_maps.append({
            "xT": np.ascontiguousarray(x[b].T).astype(ml_dtypes.bfloat16),
            "wqk": wqk.astype(ml_dtypes.bfloat16),
            "wv": wv.astype(ml_dtypes.bfloat16),
            "wout": wout.astype(ml_dtypes.bfloat16),
            "csn": csn, "mk": mk,
        })
    return in_maps


def _get_runtime(repeat=1):
    """Compile once; return a cached sharded jitted callable + metadata."""
    key = ("rt", repeat)
    if key in _CACHE:
        return _CACHE[key]
    import jax
    import numpy as _np
    from jax.sharding import Mesh, PartitionSpec
    from jax.experimental.shard_map import shard_map
    import concourse.mybir as mybir
    from concourse import bass2jax

    nc = _build_program(repeat=repeat)
    bass2jax.install_neuronx_cc_hook()

    partition_name = (nc.partition_id_tensor.name
                      if nc.partition_id_tensor else None)
    in_names, out_names, out_avals, zero_outs = [], [], [], []
    for alloc in nc.m.functions[0].allocations:
        if not isinstance(alloc, mybir.MemoryLocationSet):
            continue
        name = alloc.memorylocations[0].name
        if alloc.kind == "ExternalInput":
            if name != partition_name:
                in_names.append(name)
        elif alloc.kind == "ExternalOutput":
            np_dt = mybir.dt.np(alloc.dtype)
            out_names.append(name)
            out_avals.append(jax.core.ShapedArray(tuple(alloc.tensor_shape), np_dt))
            zero_outs.append(_np.zeros(tuple(alloc.tensor_shape), np_dt))

    n_params = len(in_names)
    n_outs = len(out_names)
    all_in_names = list(in_names) + list(out_names)
    if partition_name is not None:
        all_in_names.append(partition_name)
    donate = tuple(range(n_params, n_params + n_outs))

    def _body(*args):
        operands = list(args)
        if partition_name is not None:
            operands.append(bass2jax.partition_id_tensor())
        outs = bass2jax._bass_exec_p.bind(
            *operands,
            out_avals=tuple(out_avals),
            in_names=tuple(all_in_names),
            out_names=tuple(out_names),
            lowering_input_output_aliases=(),
            sim_require_finite=True,
            sim_require_nnan=True,
            nc=nc,
        )
        return tuple(outs)

    devices = jax.devices()[:N_CORES]
    mesh = Mesh(np.asarray(devices), ("core",))
    in_specs = (PartitionSpec("core"),) * (n_params + n_outs)
    out_specs = (PartitionSpec("core"),) * n_outs
    fn = jax.jit(
        shard_map(_body, mesh=mesh, in_specs=in_specs, out_specs=out_specs,
                  check_rep=False),
        donate_argnums=donate, keep_unused=True)

    rt = dict(fn=fn, in_names=in_names, out_names=out_names,
              zero_outs=zero_outs, mesh=mesh)
    _CACHE[key] = rt
    return rt


def _run(in_maps):
    rt = _get_runtime()
    concat_in = [np.concatenate([np.asarray(in_maps[c][n]) for c in range(N_CORES)],
                                axis=0) for n in rt["in_names"]]
    concat_zeros = [np.zeros((N_CORES * z.shape[0], *z.shape[1:]), z.dtype)
                    for z in rt["zero_outs"]]
    out_arrs = rt["fn"](*concat_in, *concat_zeros)
    y_all = np.asarray(out_arrs[0]).reshape(N_CORES, T, C)
    return y_all


def kernel(x, W_qkv, W_out):
    in_maps = _host_inputs(x, W_qkv, W_out)
    y_all = _run(in_maps)
    y = np.empty((B, T, C), dtype=np.float32)
    for b in range(B):
        y[b] = y_all[2 * b] + y_all[2 * b + 1]
    return y

